# revision 1
# baseline (speedup 1.0000x reference)
"""Grouped (MoE-routed) GEMM on 8 Trainium2 NeuronCores.

out[m, n] = sum_k lhs[m, k] * rhs[g[m], n, k],  g = clamp(m_indices, 0, G)

Strategy: expert-parallel. Host dispatches rows by m_indices (the
"all-to-all" is a host-side gather since we hold full inputs), core c gets
expert c's rows padded to a common M_pad, plus expert c's weight matrix.
Every core then runs one identical dense GEMM program computing the
transposed output:

    oT[N, M_pad] = B[N, K] @ A[M_pad, K]^T    (bf16 in, fp32 accum, bf16 out)

Both operands are pre-transposed to K-major on the host (shard-prep
layout), so the device only issues full-rate contiguous DMAs — no
on-device transposes. B tiles are the stationary operand, A the moving
operand. Each PSUM tile accumulates one (n-tile, m-chunk) over the 8
K-chunks; chains are emitted m-chunk-outer so the PE can start once the
first m-column of A and the first n-tile of B have landed. A few dummy
matmuls on a scratch tile run during the load window to lift the PE HAM
clock gate before real work arrives. Loads are split across both HWDGE
rings (SP + ACT), ordered first-needed-first; stores go out through the
otherwise-idle GpSimd SWDGE path. DVE casts PSUM -> SBUF bf16.
"""

import numpy as np
import ml_dtypes

K = 1024
N = 2048
G = 8
N_CORES = 8
KP = 128           # SBUF partitions / contraction chunk
KC = K // KP       # 8 k-chunks
MCH = 512          # max moving-operand m-chunk (one PSUM bank of fp32)
N_WARMUP = 10      # dummy matmuls to lift the HAM clock gate

_BUILD_CACHE = {}


def _m_chunks(m_pad):
    """Split m_pad into near-equal 64-aligned chunks of <=512.

    Near-equal (rather than 512,512,...,remainder) keeps every chunk wide
    enough that the per-chain LDWEIGHTS stays hidden under the matmuls.
    """
    n_chunks = -(-m_pad // MCH)
    base = m_pad // n_chunks // 64 * 64
    sizes = [base] * n_chunks
    extra = m_pad - base * n_chunks
    i = 0
    while extra > 0:
        add = min(64, extra)
        sizes[i] += add
        extra -= add
        i = (i + 1) % n_chunks
    chunks = []
    m = 0
    for w in sizes:
        chunks.append((m, w))
        m += w
    assert m == m_pad and all(w <= MCH for _, w in chunks)
    return chunks


def _build(m_pad):
    import concourse.mybir as mybir
    import concourse.tile as tile
    from concourse import bacc

    if m_pad in _BUILD_CACHE:
        return _BUILD_CACHE[m_pad]

    nc = bacc.Bacc("TRN2", target_bir_lowering=False, debug=False,
                   num_devices=N_CORES)

    at_d = nc.dram_tensor("at", [KC, KP, m_pad], mybir.dt.bfloat16,
                          kind="ExternalInput")
    bt_d = nc.dram_tensor("bt", [KC, KP, N], mybir.dt.bfloat16,
                          kind="ExternalInput")
    o_d = nc.dram_tensor("o", [N, m_pad], mybir.dt.bfloat16,
                         kind="ExternalOutput")

    nt_n = N // KP           # 16 stationary n-tiles
    chunks = _m_chunks(m_pad)

    def ring(i):
        return nc.sync if i % 2 == 0 else nc.scalar

    with tile.TileContext(nc) as tc:
        with (
            tc.tile_pool(name="ats", bufs=1) as ap,
            tc.tile_pool(name="bts", bufs=1) as bp,
            tc.tile_pool(name="wrm", bufs=1) as wp,
            tc.tile_pool(name="ost", bufs=4) as op,
            tc.tile_pool(name="ps", bufs=8, space="PSUM") as pp,
        ):
            at_s = ap.tile([KP, KC, m_pad], mybir.dt.bfloat16)
            bt_s = bp.tile([KP, KC, N], mybir.dt.bfloat16)

            # PE warmup: junk matmuls (scratch tile) run while input DMAs
            # stream, so the HAM clock gate is released (1.2 -> 2.4 GHz)
            # before the first real matmul.
            if N_WARMUP:
                junk = wp.tile([KP, MCH], mybir.dt.bfloat16)
                nc.gpsimd.memset(junk[:], 0.0)
                wps = pp.tile([KP, MCH], mybir.dt.float32, name="wps",
                              tag="ps")
                for _ in range(N_WARMUP):
                    nc.tensor.matmul(wps[:], junk[:, 0:KP], junk[:],
                                     start=True, stop=True)

            # Loads, first-needed-first, alternating rings by k-chunk.
            # The first compute column (m-chunk 0 swept down all n-tiles)
            # needs at[ci0] + bt[nt] progressively; B n-tile supply
            # (0.63us/tile at ~400GB/s) outpaces PE consumption
            # (1.4us/chain), so after the first chain the PE should never
            # starve. Granules stay >=64KB to keep DMA issue cost small.
            (mc0, w0) = chunks[0]
            for kc in range(KC):
                ring(kc).dma_start(bt_s[:, kc, 0:2 * KP],
                                   bt_d[kc, :, 0:2 * KP])
                ring(kc).dma_start(at_s[:, kc, mc0:mc0 + w0],
                                   at_d[kc, :, mc0:mc0 + w0])
            for kc in range(KC):
                ring(kc).dma_start(bt_s[:, kc, 2 * KP:8 * KP],
                                   bt_d[kc, :, 2 * KP:8 * KP])
            for kc in range(KC):
                ring(kc).dma_start(bt_s[:, kc, 8 * KP:N],
                                   bt_d[kc, :, 8 * KP:N])
            for (mc, w) in chunks[1:]:
                for kc in range(KC):
                    ring(kc).dma_start(at_s[:, kc, mc:mc + w],
                                       at_d[kc, :, mc:mc + w])

            # GEMM: one PSUM accumulation chain per (n-tile, m-chunk),
            # m-chunk-outer so the first column starts on minimal data.
            for ci, (mc, w) in enumerate(chunks):
                last_col = ci == len(chunks) - 1
                for nt in range(nt_n):
                    p = pp.tile([KP, w], mybir.dt.float32, name="p",
                                tag="ps")
                    for kc in range(KC):
                        nc.tensor.matmul(
                            p[:],
                            bt_s[:, kc, nt * KP:(nt + 1) * KP],
                            at_s[:, kc, mc:mc + w],
                            start=(kc == 0),
                            stop=(kc == KC - 1),
                        )
                    ot = op.tile([KP, w], mybir.dt.bfloat16, name="ot")
                    nc.vector.tensor_copy(ot[:], p[:])
                    # Bulk stores ride the otherwise-idle SWDGE path so the
                    # HWDGE rings stay clear for loads; the final column
                    # goes out via the (by now idle) HWDGE rings instead,
                    # so the slow GpSimd queue drain at kernel exit overlaps
                    # with compute rather than extending the tail.
                    st = ring(nt) if last_col else nc.gpsimd
                    st.dma_start(
                        o_d[nt * KP:(nt + 1) * KP, mc:mc + w], ot[:])

    nc.compile()
    _BUILD_CACHE[m_pad] = nc
    return nc


SEC_CAP = 4096     # max rows one core takes in one launch (bounds SBUF use)


def _shard(m_indices):
    """Dispatch rows to (expert, row-subset) sections, <=8 per launch.

    In the common balanced case this is exactly one section per expert and
    a single launch. If one expert is so heavy that its section exceeds
    SEC_CAP, it is split into multiple sections (and, beyond 8 sections
    total, into multiple launches) so SBUF capacity is never exceeded.
    """
    g = np.where((m_indices >= 0) & (m_indices < G), m_indices, 0)
    rows = [np.nonzero(g == e)[0] for e in range(G)]
    sections = []                        # (expert, row_indices)
    for e in range(G):
        for s in range(0, max(len(rows[e]), 1), SEC_CAP):
            sections.append((e, rows[e][s:s + SEC_CAP]))
    sections.sort(key=lambda s: -len(s[1]))
    launches = [sections[i:i + N_CORES]
                for i in range(0, len(sections), N_CORES)]
    return launches


def _prep_in_maps(lhs, rhs, launch, m_pad):
    in_maps = []
    for slot in range(N_CORES):
        e, r = launch[slot] if slot < len(launch) else (0, [])
        a = np.zeros((m_pad, K), dtype=ml_dtypes.bfloat16)
        if len(r):
            a[:len(r)] = lhs[r]
        at = a.T.reshape(KC, KP, m_pad)          # [k, m] -> [kc, kp, m]
        bt = rhs[e].T.reshape(KC, KP, N)         # [n, k] -> [kc, kp, n]
        in_maps.append({
            "at": np.ascontiguousarray(at),
            "bt": np.ascontiguousarray(bt),
        })
    return in_maps


def kernel(lhs, rhs, m_indices):
    from concourse import bass_utils

    lhs = np.asarray(lhs)
    rhs = np.asarray(rhs)
    m_indices = np.asarray(m_indices)
    M = lhs.shape[0]

    out = np.zeros((M, N), dtype=ml_dtypes.bfloat16)
    for launch in _shard(m_indices):
        m_pad = max(-(-max(len(r) for _, r in launch) // 64) * 64, 128)
        nc = _build(m_pad)
        in_maps = _prep_in_maps(lhs, rhs, launch, m_pad)
        res = bass_utils.run_bass_kernel_spmd(
            nc, in_maps, core_ids=list(range(N_CORES)))
        for slot, (e, r) in enumerate(launch):
            if len(r):
                oT = res.results[slot]["o"]      # [N, m_pad]
                out[r] = oT[:, :len(r)].T
    return out

